# revision 17
# baseline (speedup 1.0000x reference)
"""Trainium2 Bass kernel for a pre-LN decoder block (B=2, T=2048, E=1024,
H=16, HD=64, FF=4096), run SPMD across 8 NeuronCores.

v2 (KV AllGather + streaming attention):
  - Tokens are sequence-sharded: core c owns batch c//4 and the interleaved
    token set {t : t % 4 == c % 4} (512 tokens).  Unlike v1, each core
    computes K/V only for its OWN tokens; the 4 cores of a batch group
    exchange K/V via one AllGather (groups [[0..3],[4..7]]).  This removes
    3/4 of the K/V projection matmuls, 3/4 of LN1, and the full-batch x
    DMA.
  - Scores are computed j-chunk-stationary: one matmul per (pair, sub,
    section, chunk) streaming all valid q-tiles (N up to 512) instead of
    per-(q-tile, chunk) 128-wide matmuls.  Cuts PE weight-load overhead.
  - AV streams the same way: per (head, section, chunk) one matmul over
    the valid q-range, accumulating into a single [65, 512] PSUM tile with
    split stop-calls on the last section.
  - exp runs on the scalar engine over both subs of a [128, 2, N] PSUM
    strip in one call; softmax reciprocal uses reciprocal_approx_fast.
  - LN uses bn_stats/bn_aggr (one DVE pass for mean+var).
"""

import math
import numpy as np
import ml_dtypes

import concourse.bass as bass
import concourse.tile as tile
from concourse import bacc, mybir
from concourse.bass_utils import run_bass_kernel_spmd

F32 = mybir.dt.float32
BF16 = mybir.dt.bfloat16
AF = mybir.ActivationFunctionType
ALU = mybir.AluOpType

B, T, E, H, HD, FF = 2, 2048, 1024, 16, 64, 4096
LN_EPS = 1e-5
SCALE = E ** -0.5          # NOTE: reference scales by E**-0.5, not HD**-0.5
NCORES = 8
TQ = 512                   # own tokens per core
NP = H // 2                # head pairs
EC = E // 128              # e-chunks
QT = TQ // 128             # q tiles (4)
FT = FF // 128             # ff tiles (32)
NSEC = 4                   # sections (cores per batch group)
CH = T // 128              # total kv chunks (16)

# sexp slot offsets: per section s, chunk cj covers q-cols [cj*128, 512);
# widths 512,384,256,128 -> cumulative offsets within a section
CUM = [0, 512, 896, 1152]
SECW = 1280                # total sexp cols per section
SEXPW = NSEC * SECW        # 5120 per sub

KV_K = 8                   # kv_in slots 0..7  : K pairs   [128, 512]
KV_V0 = 8                  # kv_in slots 8..15 : V chunks  (2 slots each)


def _layernorm(nc, tc, pool_s, xa, eps_sb, out_bf, n_free):
    """LN (no affine) of xa [128, n_free] f32 -> out_bf [128, n_free] bf16.
    Uses bn_stats/bn_aggr; scalar does sqrt + final normalize."""
    nsub = n_free // 512
    stats = pool_s.tile([128, nsub, 6], F32, tag="stats", name="stats")
    xview = xa.rearrange("p (n c) -> p n c", n=nsub)
    for g in range(nsub):
        nc.vector.bn_stats(stats[:, g, :], xview[:, g, :])
    mv = pool_s.tile([128, 2], F32, tag="mv", name="mv")
    nc.vector.bn_aggr(mv[:], stats[:])
    std = pool_s.tile([128, 1], F32, tag="std", name="std")
    nc.scalar.activation(std[:], mv[:, 1:2], AF.Sqrt, bias=eps_sb[:])
    rstd = pool_s.tile([128, 1], F32, tag="rstd", name="rstd")
    nc.vector.reciprocal(rstd[:], std[:])
    nmr = pool_s.tile([128, 1], F32, tag="nmr", name="nmr")
    nc.vector.tensor_scalar(nmr[:], mv[:, 0:1], rstd[:], -1.0,
                            ALU.mult, ALU.mult)
    nc.scalar.activation(out_bf, xa, AF.Identity, scale=rstd[:], bias=nmr[:])


def build_module(apply_beta1=False, apply_beta2=False, debug_taps=False):
    nc = bacc.Bacc("TRN2", target_bir_lowering=False, debug=False,
                   enable_asserts=True, num_devices=NCORES)
    taps = {}
    if debug_taps:
        taps["hT"] = nc.dram_tensor("d_hT", [128, EC, TQ], BF16, kind="ExternalOutput").ap()
        taps["kT0"] = nc.dram_tensor("d_kT0", [128, T], BF16, kind="ExternalOutput").ap()
        taps["qT0"] = nc.dram_tensor("d_qT0", [128, TQ], BF16, kind="ExternalOutput").ap()
        taps["vS0"] = nc.dram_tensor("d_vS0", [128, 16, 65], BF16, kind="ExternalOutput").ap()
        taps["sexp0"] = nc.dram_tensor("d_sexp0", [128, 2, SEXPW], BF16, kind="ExternalOutput").ap()
        taps["oTn0"] = nc.dram_tensor("d_oTn0", [128, QT, 128], BF16, kind="ExternalOutput").ap()
        taps["x2_0"] = nc.dram_tensor("d_x2_0", [128, E], F32, kind="ExternalOutput").ap()

    x_own = nc.dram_tensor("x_own", [TQ, E], F32, kind="ExternalInput").ap()
    x_res = nc.dram_tensor("x_res", [TQ, E], F32, kind="ExternalInput").ap()
    wq = nc.dram_tensor("wq", [E, E], BF16, kind="ExternalInput").ap()
    wk = nc.dram_tensor("wk", [E, E], BF16, kind="ExternalInput").ap()
    wv = nc.dram_tensor("wv", [E, E], BF16, kind="ExternalInput").ap()
    wp = nc.dram_tensor("wp", [E, E], BF16, kind="ExternalInput").ap()
    w1 = nc.dram_tensor("w1", [E, FF], BF16, kind="ExternalInput").ap()
    w2 = nc.dram_tensor("w2", [FF, E], BF16, kind="ExternalInput").ap()
    b1c = nc.dram_tensor("b1c", [128, FT], F32, kind="ExternalInput").ap()
    be1c = nc.dram_tensor("be1c", [128, EC], F32, kind="ExternalInput").ap()
    be2c = nc.dram_tensor("be2c", [128, EC], F32, kind="ExternalInput").ap()
    maskd = nc.dram_tensor("maskd", [128, NSEC, 128], BF16, kind="ExternalInput").ap()
    identd = nc.dram_tensor("identd", [128, 128], BF16, kind="ExternalInput").ap()
    y = nc.dram_tensor("y", [TQ, E], F32, kind="ExternalOutput").ap()

    import contextlib
    with tile.TileContext(nc) as tc, contextlib.ExitStack() as st:
        # ---------- persistent SBUF tiles ----------
        pers = st.enter_context(tc.tile_pool(name="pers", bufs=1))
        kT = [pers.tile([128, T], BF16, tag=f"kT{p}", name=f"kT{p}") for p in range(NP)]
        qT = [pers.tile([128, TQ], BF16, tag=f"qT{p}", name=f"qT{p}") for p in range(NP)]
        vS = [pers.tile([128, 16, 65], BF16, tag=f"vS{j}", name=f"vS{j}") for j in range(CH)]
        oTn = [pers.tile([128, QT, 128], BF16, tag=f"oTn{p}", name=f"oTn{p}")
               for p in range(NP)]
        x2 = [pers.tile([128, E], F32, tag=f"x2{tq}", name=f"x2{tq}")
              for tq in range(QT)]
        h2A = pers.tile([128, EC, TQ], BF16, tag="h2A", name="h2A")
        consts = st.enter_context(tc.tile_pool(name="consts", bufs=1))
        b1_sb = consts.tile([128, FT], F32, tag="b1", name="b1")
        mask_sb = consts.tile([128, NSEC, 128], BF16, tag="mask", name="mask")
        eps_sb = consts.tile([128, 1], F32, tag="eps", name="eps")
        ident_sb = consts.tile([128, 128], BF16, tag="ident", name="ident")
        nc.sync.dma_start(ident_sb[:], identd[:])
        nc.gpsimd.memset(eps_sb[:], LN_EPS)
        nc.sync.dma_start(b1_sb[:], b1c[:])
        nc.sync.dma_start(mask_sb[:], maskd[:])
        for j in range(CH):
            nc.gpsimd.memset(vS[j][:, :, 64:65], 1.0)
        if apply_beta1:
            be1_sb = consts.tile([128, EC], F32, tag="be1", name="be1")
            nc.sync.dma_start(be1_sb[:], be1c[:])
        if apply_beta2:
            be2_sb = consts.tile([128, EC], F32, tag="be2", name="be2")
            nc.sync.dma_start(be2_sb[:], be2c[:])

        # DRAM bounce buffers for the K/V AllGather (whole-kernel scope).
        # The gather runs over all 8 ranks (4-rank groups fall into a slow
        # ring algorithm, ~130us; 8-rank uses the fast path, ~20us).  Each
        # core reads back only its batch group's 4 rows via a dynamic
        # (partition_id & 4) row offset so the NEFF stays SPMD-uniform.
        cc = st.enter_context(tc.tile_pool(name="cc", bufs=1, space="DRAM"))
        kv_in = cc.tile([16, 128, 512], BF16, tag="kv_in", name="kv_in")
        kv_out = cc.tile([NCORES, 16, 128, 512], BF16, tag="kv_out",
                         name="kv_out", addr_space="Shared")

        # ---------- phase 1+2: LN1 (own), K/V/Q proj (own), AllGather ----------
        with tc.tile_pool(name="wqkv", bufs=1) as wqkv, \
             tc.tile_pool(name="hTp", bufs=1) as hTp, \
             tc.tile_pool(name="kvstg", bufs=1) as kvstg, \
             tc.tile_pool(name="p1x", bufs=3) as p1x, \
             tc.tile_pool(name="p1s", bufs=4) as p1s:
            wk_sb = [wqkv.tile([128, E], BF16, tag=f"wk{ec}", name=f"wk{ec}") for ec in range(EC)]
            wv_sb = [wqkv.tile([128, E], BF16, tag=f"wv{ec}", name=f"wv{ec}") for ec in range(EC)]
            wq_sb = [wqkv.tile([128, E], BF16, tag=f"wq{ec}", name=f"wq{ec}") for ec in range(EC)]
            for ec in range(EC):
                nc.sync.dma_start(wk_sb[ec][:], wk[ec * 128:(ec + 1) * 128, :])
            for ec in range(EC):
                nc.sync.dma_start(wv_sb[ec][:], wv[ec * 128:(ec + 1) * 128, :])
            for ec in range(EC):
                nc.sync.dma_start(wq_sb[ec][:], wq[ec * 128:(ec + 1) * 128, :])
            hT = hTp.tile([128, EC, TQ], BF16, tag="hT", name="hT")
            kstage = [kvstg.tile([128, TQ], BF16, tag=f"ks{p}", name=f"ks{p}")
                      for p in range(NP)]
            vstage = [kvstg.tile([128, 16, 64], BF16, tag=f"vs{c}", name=f"vs{c}")
                      for c in range(QT)]

            with tc.tile_pool(name="pT1", bufs=2, space="PSUM") as pT1, \
                 tc.tile_pool(name="p2k", bufs=2, space="PSUM") as p2k, \
                 tc.tile_pool(name="p2v", bufs=2, space="PSUM") as p2v, \
                 tc.tile_pool(name="p2q", bufs=2, space="PSUM") as p2q:
                # LN1 over own 4 token tiles
                for i2 in range(QT):
                    xa = p1x.tile([128, E], F32, tag="xa", name="xa")
                    nc.sync.dma_start(xa[:], x_own[i2 * 128:(i2 + 1) * 128, :])
                    hnat = p1x.tile([128, E], BF16, tag="hnat", name="hnat")
                    _layernorm(nc, tc, p1s, xa[:], eps_sb, hnat[:], E)
                    pt = pT1.tile([128, EC, 128], BF16, tag="pt", name="pt")
                    for ec in range(EC):
                        nc.tensor.transpose(pt[:, ec, :],
                                            hnat[:, ec * 128:(ec + 1) * 128],
                                            ident_sb[:])
                    nc.scalar.copy(hT[:, :, i2 * 128:(i2 + 1) * 128], pt[:])
                if apply_beta1:
                    for ec in range(EC):
                        nc.vector.tensor_scalar_add(
                            hT[:, ec, :], hT[:, ec, :], be1_sb[:, ec:ec + 1])

                # K projection (own tokens) -> kstage -> kv_in
                for p in range(NP):
                    psk = p2k.tile([128, TQ], F32, tag="psk", name="psk")
                    for ec in range(EC):
                        nc.tensor.matmul(psk[:],
                                         wk_sb[ec][:, p * 128:(p + 1) * 128],
                                         hT[:, ec, :],
                                         start=(ec == 0), stop=(ec == EC - 1))
                    nc.scalar.copy(kstage[p][:], psk[:])
                    nc.gpsimd.dma_start(kv_in[p, :, :], kstage[p][:])
                # V projection (own tokens) -> vstage -> kv_in
                for c in range(QT):
                    for half in range(2):
                        psv = p2v.tile([128, 8, 64], F32, tag="psv", name="psv")
                        for ec in range(EC):
                            nc.tensor.matmul(
                                psv[:],
                                hT[:, ec, c * 128:(c + 1) * 128],
                                wv_sb[ec][:, half * 512:(half + 1) * 512],
                                start=(ec == 0), stop=(ec == EC - 1))
                        nc.scalar.copy(vstage[c][:, half * 8:(half + 1) * 8, :], psv[:])
                    for half in range(2):
                        nc.gpsimd.dma_start(
                            kv_in[KV_V0 + 2 * c + half, :, :]
                            .rearrange("p (h d) -> p h d", d=64),
                            vstage[c][:, half * 8:(half + 1) * 8, :])

                # AllGather K+V across all 8 ranks (fast path)
                nc.gpsimd.collective_compute(
                    "AllGather", mybir.AluOpType.bypass,
                    replica_groups=[[0, 1, 2, 3, 4, 5, 6, 7]],
                    ins=[kv_in[:].opt()],
                    outs=[kv_out[:].opt()],
                )
                # readback own batch group's rows, section-major so scores
                # can chase; row = (partition_id & 4) + s at runtime
                grp = nc.gpsimd.partition_id() & 4
                for s in range(NSEC):
                    row = bass.ds(grp + s, 1)
                    for p in range(NP):
                        nc.gpsimd.dma_start(kT[p][:, s * 512:(s + 1) * 512],
                                            kv_out[row, p, :, :])
                    for c in range(QT):
                        for half in range(2):
                            nc.gpsimd.dma_start(
                                vS[s * 4 + c][:, half * 8:(half + 1) * 8, 0:64],
                                kv_out[row, KV_V0 + 2 * c + half, :, :]
                                .rearrange("a p (h d) -> a p h d", d=64))

                # Q projection (own tokens)
                for p in range(NP):
                    psq = p2q.tile([128, TQ], F32, tag="psq", name="psq")
                    for ec in range(EC):
                        nc.tensor.matmul(psq[:],
                                         wq_sb[ec][:, p * 128:(p + 1) * 128],
                                         hT[:, ec, :],
                                         start=(ec == 0), stop=(ec == EC - 1))
                    nc.scalar.copy(qT[p][:], psq[:])
                if debug_taps:
                    nc.sync.dma_start(taps["hT"][:], hT[:])

        # ---------- phases 3-5 ----------
        with tc.tile_pool(name="p4wp", bufs=1) as p4wp, \
             tc.tile_pool(name="p5w", bufs=24) as p5w:
            wp_sb = [p4wp.tile([128, E], BF16, tag=f"wp{p}", name=f"wp{p}")
                     for p in range(NP)]
            for p in range(NP):
                nc.sync.dma_start(wp_sb[p][:], wp[p * 128:(p + 1) * 128, :])
            w1_tiles = {}
            FBLK = 4  # f-tiles per W1 block

            def load_w1_block(fb):
                for ec in range(EC):
                    t = p5w.tile([128, FBLK * 128], BF16, tag="w1c", name="w1c")
                    nc.sync.dma_start(
                        t[:], w1[ec * 128:(ec + 1) * 128,
                                 fb * FBLK * 128:(fb + 1) * FBLK * 128])
                    w1_tiles[(fb, ec)] = t

            load_w1_block(0)
            load_w1_block(1)

            # ---------- phase 3: attention ----------
            with tc.tile_pool(name="p3sexp", bufs=2) as p3sexp, \
                 tc.tile_pool(name="p3strip", bufs=2, space="PSUM") as p3strip, \
                 tc.tile_pool(name="p3oT", bufs=2, space="PSUM") as p3oT, \
                 tc.tile_pool(name="p3sm", bufs=4) as p3sm:

                sexp_of = {}

                def emit_scores(p):
                    sexp = p3sexp.tile([128, 2, SEXPW], BF16, tag="sexp", name="sexp")
                    sexp_of[p] = sexp
                    for s in range(NSEC):
                        for cj in range(QT):
                            n = (4 - cj) * 128
                            off = s * SECW + CUM[cj]
                            strip = p3strip.tile([128, 2, 512], F32, tag="strip",
                                                 name="strip")
                            for sub in range(2):
                                nc.tensor.matmul(
                                    strip[:, sub, 0:n],
                                    kT[p][sub * 64:(sub + 1) * 64,
                                          s * 512 + cj * 128:s * 512 + (cj + 1) * 128],
                                    qT[p][sub * 64:(sub + 1) * 64, cj * 128:512],
                                    start=True, stop=True)
                            nc.scalar.activation(sexp[:, :, off:off + n],
                                                 strip[:, :, 0:n], AF.Exp,
                                                 scale=SCALE)
                            # causal mask on the diagonal q-tile (i == cj)
                            for sub in range(2):
                                nc.vector.tensor_tensor(
                                    sexp[:, sub, off:off + 128],
                                    sexp[:, sub, off:off + 128],
                                    mask_sb[:, s, :], ALU.mult)

                def emit_av(p):
                    sexp = sexp_of.pop(p)
                    for sub in range(2):
                        h = 2 * p + sub
                        oT = p3oT.tile([65, 512], F32, tag="oT", name="oT")
                        # PSUM accumulation is per-element has_written; the
                        # first call spans the whole bank so one start/stop
                        # pair suffices for the wide streaming calls.
                        for s in range(NSEC):
                            for cj in range(QT):
                                n = (4 - cj) * 128
                                off = s * SECW + CUM[cj]
                                nc.tensor.matmul(
                                    oT[:, cj * 128:512],
                                    vS[s * 4 + cj][:, h, :],
                                    sexp[:, sub, off:off + n],
                                    start=(s == 0 and cj == 0),
                                    stop=(s == NSEC - 1 and cj == QT - 1))
                        # softmax normalize + drain
                        sums = p3sm.tile([1, TQ], F32, tag="sums", name="sums")
                        nc.vector.tensor_copy(sums[:], oT[64:65, :])
                        nc.vector.reciprocal(sums[:], sums[:])
                        rb = p3sm.tile([128, TQ], F32, tag="rb", name="rb")
                        nc.gpsimd.partition_broadcast(rb[:], sums[:])
                        rbh = rb[sub * 64:sub * 64 + 64, :]
                        dst = oTn[p][sub * 64:sub * 64 + 64, :, :] \
                            .rearrange("p a b -> p (a b)")
                        nc.vector.tensor_copy(dst, oT[0:64, :])
                        nc.vector.tensor_tensor(dst, dst, rbh, ALU.mult)

                for p in range(NP):
                    emit_scores(p)
                    if debug_taps and p == 0:
                        nc.sync.dma_start(taps["sexp0"][:], sexp_of[0][:])
                    if p >= 1:
                        emit_av(p - 1)
                emit_av(NP - 1)
                if debug_taps:
                    nc.sync.dma_start(taps["kT0"][:], kT[0][:])
                    nc.sync.dma_start(taps["qT0"][:], qT[0][:])
                    nc.sync.dma_start(taps["vS0"][:], vS[0][:])
                    nc.sync.dma_start(taps["oTn0"][:], oTn[0][:])

            # ---------- phase 4: proj + residual + LN2 + transpose ----------
            with tc.tile_pool(name="p4xr", bufs=2) as p4xr, \
                 tc.tile_pool(name="p4ps", bufs=3, space="PSUM") as p4ps, \
                 tc.tile_pool(name="pT4", bufs=2, space="PSUM") as pT4, \
                 tc.tile_pool(name="p4s", bufs=4) as p4s, \
                 tc.tile_pool(name="p4h", bufs=3) as p4h:
                for tq in range(QT):
                    xr = p4xr.tile([128, E], F32, tag="xr", name="xr")
                    nc.sync.dma_start(xr[:], x_res[tq * 128:(tq + 1) * 128, :])
                    for half in range(2):
                        ps = p4ps.tile([128, 512], F32, tag="pproj", name="pproj")
                        for p in range(NP):
                            nc.tensor.matmul(ps[:], oTn[p][:, tq, :],
                                             wp_sb[p][:, half * 512:(half + 1) * 512],
                                             start=(p == 0), stop=(p == NP - 1))
                        nc.vector.tensor_add(x2[tq][:, half * 512:(half + 1) * 512],
                                             ps[:], xr[:, half * 512:(half + 1) * 512])
                    hnat = p4h.tile([128, E], BF16, tag="hnat", name="hnat")
                    _layernorm(nc, tc, p4s, x2[tq][:], eps_sb, hnat[:], E)
                    pt = pT4.tile([128, EC, 128], BF16, tag="pt", name="pt")
                    for ec in range(EC):
                        nc.tensor.transpose(pt[:, ec, :],
                                            hnat[:, ec * 128:(ec + 1) * 128],
                                            ident_sb[:])
                    nc.scalar.copy(h2A[:, :, tq * 128:(tq + 1) * 128], pt[:])
                if apply_beta2:
                    for ec in range(EC):
                        nc.vector.tensor_scalar_add(h2A[:, ec, :], h2A[:, ec, :],
                                                    be2_sb[:, ec:ec + 1])
                if debug_taps:
                    nc.sync.dma_start(taps["x2_0"][:], x2[0][:])

            # ---------- phase 5: FFN ----------
            with tc.tile_pool(name="p5g", bufs=1) as p5g, \
                 tc.tile_pool(name="p5w2", bufs=16) as p5w2, \
                 tc.tile_pool(name="p5pg", bufs=3, space="PSUM") as p5pg, \
                 tc.tile_pool(name="p5py", bufs=4, space="PSUM") as p5py, \
                 tc.tile_pool(name="p5o", bufs=3) as p5o:
                gT = [p5g.tile([128, TQ], BF16, tag=f"gT{f}", name=f"gT{f}")
                      for f in range(FT)]
                for fb in range(FT // FBLK):
                    for fi in range(FBLK):
                        f = fb * FBLK + fi
                        ps = p5pg.tile([128, TQ], F32, tag="pg", name="pg")
                        for ec in range(EC):
                            nc.tensor.matmul(
                                ps[:], w1_tiles[(fb, ec)][:, fi * 128:(fi + 1) * 128],
                                h2A[:, ec, :], start=(ec == 0), stop=(ec == EC - 1))
                        nc.scalar.activation(gT[f][:], ps[:], AF.Relu,
                                             bias=b1_sb[:, f:f + 1])
                    if fb + 2 < FT // FBLK and fb + 2 >= 2:
                        load_w1_block(fb + 2)
                for half in range(2):
                    psy = [p5py.tile([128, 512], F32, tag="py", name="py")
                           for _ in range(QT)]
                    for f in range(FT):
                        w2c = p5w2.tile([128, 512], BF16, tag="w2c", name="w2c")
                        nc.sync.dma_start(
                            w2c[:], w2[f * 128:(f + 1) * 128,
                                       half * 512:(half + 1) * 512])
                        for tq in range(QT):
                            nc.tensor.matmul(psy[tq][:],
                                             gT[f][:, tq * 128:(tq + 1) * 128],
                                             w2c[:], start=(f == 0),
                                             stop=(f == FT - 1))
                    for tq in range(QT):
                        outsb = p5o.tile([128, 512], F32, tag="outsb", name="outsb")
                        nc.vector.tensor_add(outsb[:], psy[tq][:],
                                             x2[tq][:, half * 512:(half + 1) * 512])
                        nc.sync.dma_start(
                            y[tq * 128:(tq + 1) * 128, half * 512:(half + 1) * 512],
                            outsb[:])

    nc.compile()
    return nc


_MODULE_CACHE = {}


def _get_module(key=(False, False)):
    if key not in _MODULE_CACHE:
        _MODULE_CACHE[key] = build_module(apply_beta1=key[0], apply_beta2=key[1])
    return _MODULE_CACHE[key]


def make_core_inputs(x, Wq, Wk, Wv, Wproj, bproj, W1, b1, W2, b2, g1, be1, g2, be2):
    """Host-side sharding/folding. Returns (in_maps, meta)."""
    bf = ml_dtypes.bfloat16
    x = np.asarray(x, np.float32)
    g1 = np.asarray(g1, np.float32)
    g2 = np.asarray(g2, np.float32)
    be1 = np.asarray(be1, np.float32)
    be2 = np.asarray(be2, np.float32)
    assert np.all(g1 != 0) and np.all(g2 != 0), "zero LN gamma unsupported"
    apply_beta1 = bool(np.any(be1 != 0))
    apply_beta2 = bool(np.any(be2 != 0))
    be1_eff = (be1 / g1).reshape(EC, 128).T.copy()
    be2_eff = (be2 / g2).reshape(EC, 128).T.copy()

    # lhsT layouts [E, (h, d)] with g folded into rows
    wq_l = (g1[:, None] * np.transpose(Wq, (1, 0, 2)).reshape(E, E)).astype(bf)
    wk_l = (g1[:, None] * np.transpose(Wk, (1, 0, 2)).reshape(E, E)).astype(bf)
    wv_l = (g1[:, None] * np.transpose(Wv, (1, 0, 2)).reshape(E, E)).astype(bf)
    wp_r = np.asarray(Wproj, np.float32).astype(bf)
    w1_l = (g2[:, None] * np.asarray(W1, np.float32)).astype(bf)
    w2_r = np.asarray(W2, np.float32).astype(bf)
    b1c = np.asarray(b1, np.float32).reshape(FT, 128).T.copy()

    in_maps = []
    for c in range(NCORES):
        b, own = c // 4, c % 4
        x_ownv = np.ascontiguousarray(x[b, own::4, :], np.float32)
        x_resid = x_ownv + np.asarray(bproj, np.float32)[None, :]
        # mask[r, s, q] = 1 if q >= r + (s > own)   (sections in physical order)
        r = np.arange(128)[:, None, None]
        sm = np.arange(NSEC)[None, :, None]
        q = np.arange(128)[None, None, :]
        mask = (q >= r + (sm > own)).astype(bf)
        in_maps.append({
            "x_own": x_ownv, "x_res": x_resid.astype(np.float32),
            "wq": wq_l, "wk": wk_l, "wv": wv_l, "wp": wp_r,
            "w1": w1_l, "w2": w2_r, "b1c": b1c,
            "be1c": be1_eff.astype(np.float32),
            "be2c": be2_eff.astype(np.float32),
            "maskd": np.ascontiguousarray(mask),
            "identd": np.eye(128, dtype=bf),
        })
    return in_maps, (apply_beta1, apply_beta2)


def assemble_output(results, b2):
    out = np.empty((B, T, E), np.float32)
    b2 = np.asarray(b2, np.float32)
    for c in range(NCORES):
        b, own = c // 4, c % 4
        out[b, own::4, :] = results[c]["y"] + b2[None, :]
    return out


def kernel(**inputs) -> np.ndarray:
    in_maps, beta_key = make_core_inputs(**inputs)
    nc = _get_module(beta_key)
    res = run_bass_kernel_spmd(nc, in_maps, core_ids=list(range(NCORES)))
    return assemble_output(res.results, inputs["b2"])


# revision 20
# speedup vs baseline: 1.5025x; 1.5025x over previous
"""Trainium2 Bass kernel for a pre-LN decoder block (B=2, T=2048, E=1024,
H=16, HD=64, FF=4096), run SPMD across 8 NeuronCores.

v4 (streaming attention, no collectives):
  - Tokens are sequence-sharded: core c owns batch c//4 and the interleaved
    token set {t : t % 4 == c % 4} (512 tokens).  Sections are host-rotated
    so every core's own tokens sit in slot 0 (one NEFF for all cores).
    Each core recomputes K/V for its whole batch: collectives in this
    environment have an ~90us floor, far more than the ~100us of extra PE
    time, and the recompute pipelines perfectly.
  - K projection for pair p is emitted inside the attention loop right
    before that pair's scores, so the PE stays busy end-to-end and the
    scalar engine's softmax exp hides underneath.
  - Scores are j-chunk-stationary: one matmul per (pair, sub, slot, chunk)
    streaming all valid q-tiles (N up to 512).  exp covers both subs of a
    [128, 2, N] PSUM strip in one scalar call.  AV streams the same way,
    accumulating into one [65, 512] PSUM tile per head (PSUM accumulation
    is per-element has_written, so one start/stop pair suffices).
  - LN uses bn_stats/bn_aggr (one DVE pass for mean+var).
"""

import math
import numpy as np
import ml_dtypes

import concourse.bass as bass
import concourse.tile as tile
from concourse import bacc, mybir
from concourse.bass_utils import run_bass_kernel_spmd

F32 = mybir.dt.float32
BF16 = mybir.dt.bfloat16
AF = mybir.ActivationFunctionType
ALU = mybir.AluOpType

B, T, E, H, HD, FF = 2, 2048, 1024, 16, 64, 4096
LN_EPS = 1e-5
SCALE = E ** -0.5          # NOTE: reference scales by E**-0.5, not HD**-0.5
NCORES = 8
TQ = 512                   # own tokens per core
NP = H // 2                # head pairs
EC = E // 128              # e-chunks
QT = TQ // 128             # q tiles (4)
FT = FF // 128             # ff tiles (32)
NSEC = 4                   # sections (cores per batch group)
CH = T // 128              # total kv chunks (16)

# sexp slot offsets: per section slot s, chunk cj covers q-cols [cj*128, 512);
# widths 512,384,256,128 -> cumulative offsets within a section
CUM = [0, 512, 896, 1152]
SECW = 1280                # total sexp cols per section
SEXPW = NSEC * SECW        # 5120 per sub


def _layernorm(nc, pool_s, xa, eps_sb, out_bf, n_free):
    """LN (no affine) of xa [128, n_free] f32 -> out_bf [128, n_free] bf16."""
    nsub = n_free // 512
    stats = pool_s.tile([128, nsub, 6], F32, tag="stats", name="stats")
    xview = xa.rearrange("p (n c) -> p n c", n=nsub)
    for g in range(nsub):
        nc.vector.bn_stats(stats[:, g, :], xview[:, g, :])
    mv = pool_s.tile([128, 2], F32, tag="mv", name="mv")
    nc.vector.bn_aggr(mv[:], stats[:])
    std = pool_s.tile([128, 1], F32, tag="std", name="std")
    nc.scalar.activation(std[:], mv[:, 1:2], AF.Sqrt, bias=eps_sb[:])
    rstd = pool_s.tile([128, 1], F32, tag="rstd", name="rstd")
    nc.vector.reciprocal(rstd[:], std[:])
    nmr = pool_s.tile([128, 1], F32, tag="nmr", name="nmr")
    nc.vector.tensor_scalar(nmr[:], mv[:, 0:1], rstd[:], -1.0,
                            ALU.mult, ALU.mult)
    nc.scalar.activation(out_bf, xa, AF.Identity, scale=rstd[:], bias=nmr[:])


def build_module(apply_beta1=False, apply_beta2=False, debug_taps=False):
    nc = bacc.Bacc("TRN2", target_bir_lowering=False, debug=False,
                   enable_asserts=True, num_devices=NCORES)
    taps = {}
    if debug_taps:
        taps["kT0"] = nc.dram_tensor("d_kT0", [128, T], BF16, kind="ExternalOutput").ap()
        taps["qT0"] = nc.dram_tensor("d_qT0", [128, TQ], BF16, kind="ExternalOutput").ap()
        taps["vS0"] = nc.dram_tensor("d_vS0", [128, 16, 65], BF16, kind="ExternalOutput").ap()
        taps["sexp0"] = nc.dram_tensor("d_sexp0", [128, 2, SEXPW], BF16, kind="ExternalOutput").ap()
        taps["oTn0"] = nc.dram_tensor("d_oTn0", [128, QT, 128], BF16, kind="ExternalOutput").ap()

    x_all = nc.dram_tensor("x_all", [T, E], F32, kind="ExternalInput").ap()
    x_res = nc.dram_tensor("x_res", [TQ, E], F32, kind="ExternalInput").ap()
    wq = nc.dram_tensor("wq", [E, E], BF16, kind="ExternalInput").ap()
    wk = nc.dram_tensor("wk", [E, E], BF16, kind="ExternalInput").ap()
    wv = nc.dram_tensor("wv", [E, E], BF16, kind="ExternalInput").ap()
    wp = nc.dram_tensor("wp", [E, E], BF16, kind="ExternalInput").ap()
    w1 = nc.dram_tensor("w1", [E, FF], BF16, kind="ExternalInput").ap()
    w2 = nc.dram_tensor("w2", [FF, E], BF16, kind="ExternalInput").ap()
    b1c = nc.dram_tensor("b1c", [128, FT], F32, kind="ExternalInput").ap()
    be1c = nc.dram_tensor("be1c", [128, EC], F32, kind="ExternalInput").ap()
    be2c = nc.dram_tensor("be2c", [128, EC], F32, kind="ExternalInput").ap()
    maskd = nc.dram_tensor("maskd", [128, NSEC, 128], BF16, kind="ExternalInput").ap()
    identd = nc.dram_tensor("identd", [128, 128], BF16, kind="ExternalInput").ap()
    y = nc.dram_tensor("y", [TQ, E], F32, kind="ExternalOutput").ap()

    import contextlib
    with tile.TileContext(nc) as tc, contextlib.ExitStack() as st:
        # ---------- persistent SBUF tiles ----------
        pers = st.enter_context(tc.tile_pool(name="pers", bufs=1))
        kT = [pers.tile([128, T], BF16, tag=f"kT{p}", name=f"kT{p}") for p in range(NP)]
        qT = [pers.tile([128, TQ], BF16, tag=f"qT{p}", name=f"qT{p}") for p in range(NP)]
        vS = [pers.tile([128, 16, 65], BF16, tag=f"vS{j}", name=f"vS{j}") for j in range(CH)]
        oTn = [pers.tile([128, QT, 128], BF16, tag=f"oTn{p}", name=f"oTn{p}")
               for p in range(NP)]
        h2A = pers.tile([128, EC, TQ], BF16, tag="h2A", name="h2A")
        wp_sb = [pers.tile([128, E], BF16, tag=f"wp{p}", name=f"wp{p}")
                 for p in range(NP)]
        consts = st.enter_context(tc.tile_pool(name="consts", bufs=1))
        b1_sb = consts.tile([128, FT], F32, tag="b1", name="b1")
        mask_sb = consts.tile([128, NSEC, 128], BF16, tag="mask", name="mask")
        eps_sb = consts.tile([128, 1], F32, tag="eps", name="eps")
        ident_sb = consts.tile([128, 128], BF16, tag="ident", name="ident")
        nc.sync.dma_start(ident_sb[:], identd[:])
        nc.gpsimd.memset(eps_sb[:], LN_EPS)
        nc.sync.dma_start(b1_sb[:], b1c[:])
        nc.sync.dma_start(mask_sb[:], maskd[:])
        for j in range(CH):
            nc.gpsimd.memset(vS[j][:, :, 64:65], 1.0)
        for p in range(NP):
            nc.sync.dma_start(wp_sb[p][:], wp[p * 128:(p + 1) * 128, :])
        if apply_beta1:
            be1_sb = consts.tile([128, EC], F32, tag="be1", name="be1")
            nc.sync.dma_start(be1_sb[:], be1c[:])
        if apply_beta2:
            be2_sb = consts.tile([128, EC], F32, tag="be2", name="be2")
            nc.sync.dma_start(be2_sb[:], be2c[:])

        # ---------- phases 1-3: LN1, V/Q proj, then K proj + attention ----------
        with tc.tile_pool(name="wkh", bufs=1) as wkh:
            wk_sb = [wkh.tile([128, E], BF16, tag=f"wk{ec}", name=f"wk{ec}")
                     for ec in range(EC)]
            hT_sec = [wkh.tile([128, EC, TQ], BF16, tag=f"hTs{s}", name=f"hTs{s}")
                      for s in range(NSEC)]
            for ec in range(EC):
                nc.sync.dma_start(wk_sb[ec][:], wk[ec * 128:(ec + 1) * 128, :])

            with tc.tile_pool(name="wvq", bufs=1) as wvq, \
                 tc.tile_pool(name="p1x", bufs=2) as p1x, \
                 tc.tile_pool(name="p1s", bufs=4) as p1s, \
                 tc.tile_pool(name="pT1", bufs=2, space="PSUM") as pT1, \
                 tc.tile_pool(name="p2v", bufs=2, space="PSUM") as p2v, \
                 tc.tile_pool(name="p2q", bufs=2, space="PSUM") as p2q:
                wv_sb = [wvq.tile([128, E], BF16, tag=f"wv{ec}", name=f"wv{ec}")
                         for ec in range(EC)]
                wq_sb = [wvq.tile([128, E], BF16, tag=f"wq{ec}", name=f"wq{ec}")
                         for ec in range(EC)]
                for ec in range(EC):
                    nc.sync.dma_start(wv_sb[ec][:], wv[ec * 128:(ec + 1) * 128, :])
                for ec in range(EC):
                    nc.sync.dma_start(wq_sb[ec][:], wq[ec * 128:(ec + 1) * 128, :])

                for sec in range(NSEC):
                    for i2 in range(4):
                        tt = sec * 4 + i2
                        xa = p1x.tile([128, E], F32, tag="xa", name="xa")
                        nc.sync.dma_start(xa[:], x_all[tt * 128:(tt + 1) * 128, :])
                        hnat = p1x.tile([128, E], BF16, tag="hnat", name="hnat")
                        _layernorm(nc, p1s, xa[:], eps_sb, hnat[:], E)
                        pt = pT1.tile([128, EC, 128], BF16, tag="pt", name="pt")
                        for ec in range(EC):
                            nc.tensor.transpose(pt[:, ec, :],
                                                hnat[:, ec * 128:(ec + 1) * 128],
                                                ident_sb[:])
                        nc.scalar.copy(hT_sec[sec][:, :, i2 * 128:(i2 + 1) * 128],
                                       pt[:])
                    if apply_beta1:
                        for ec in range(EC):
                            nc.vector.tensor_scalar_add(
                                hT_sec[sec][:, ec, :], hT_sec[sec][:, ec, :],
                                be1_sb[:, ec:ec + 1])
                    # V projection for this section's 4 chunks (chases LN)
                    for i2 in range(4):
                        vt = sec * 4 + i2
                        for half in range(2):
                            psv = p2v.tile([128, 8, 64], F32, tag="psv", name="psv")
                            for ec in range(EC):
                                nc.tensor.matmul(
                                    psv[:],
                                    hT_sec[sec][:, ec, i2 * 128:(i2 + 1) * 128],
                                    wv_sb[ec][:, half * 512:(half + 1) * 512],
                                    start=(ec == 0), stop=(ec == EC - 1))
                            nc.scalar.copy(vS[vt][:, half * 8:(half + 1) * 8, 0:64],
                                           psv[:])
                # Q projection (own tokens = slot 0)
                for p in range(NP):
                    psq = p2q.tile([128, TQ], F32, tag="psq", name="psq")
                    for ec in range(EC):
                        nc.tensor.matmul(psq[:],
                                         wq_sb[ec][:, p * 128:(p + 1) * 128],
                                         hT_sec[0][:, ec, :],
                                         start=(ec == 0), stop=(ec == EC - 1))
                    nc.scalar.copy(qT[p][:], psq[:])

            # ---------- phase 3: K proj (per pair) + attention ----------
            with tc.tile_pool(name="p3sexp", bufs=2) as p3sexp, \
                 tc.tile_pool(name="p2k", bufs=2, space="PSUM") as p2k, \
                 tc.tile_pool(name="p3strip", bufs=2, space="PSUM") as p3strip, \
                 tc.tile_pool(name="p3oT", bufs=2, space="PSUM") as p3oT, \
                 tc.tile_pool(name="p3sm", bufs=2) as p3sm:

                sexp_of = {}

                def emit_kproj(p):
                    for s in range(NSEC):
                        psk = p2k.tile([128, TQ], F32, tag="psk", name="psk")
                        for ec in range(EC):
                            nc.tensor.matmul(psk[:],
                                             wk_sb[ec][:, p * 128:(p + 1) * 128],
                                             hT_sec[s][:, ec, :],
                                             start=(ec == 0), stop=(ec == EC - 1))
                        nc.scalar.copy(kT[p][:, s * 512:(s + 1) * 512], psk[:])

                def emit_scores(p):
                    sexp = p3sexp.tile([128, 2, SEXPW], BF16, tag="sexp", name="sexp")
                    sexp_of[p] = sexp
                    for s in range(NSEC):
                        for cj in range(QT):
                            n = (4 - cj) * 128
                            off = s * SECW + CUM[cj]
                            strip = p3strip.tile([128, 2, 512], F32, tag="strip",
                                                 name="strip")
                            for sub in range(2):
                                nc.tensor.matmul(
                                    strip[:, sub, 0:n],
                                    kT[p][sub * 64:(sub + 1) * 64,
                                          s * 512 + cj * 128:s * 512 + (cj + 1) * 128],
                                    qT[p][sub * 64:(sub + 1) * 64, cj * 128:512],
                                    start=True, stop=True)
                            nc.scalar.activation(sexp[:, :, off:off + n],
                                                 strip[:, :, 0:n], AF.Exp,
                                                 scale=SCALE)
                            # causal mask on the diagonal q-tile (i == cj)
                            for sub in range(2):
                                nc.vector.tensor_tensor(
                                    sexp[:, sub, off:off + 128],
                                    sexp[:, sub, off:off + 128],
                                    mask_sb[:, s, :], ALU.mult)

                def emit_av(p):
                    sexp = sexp_of.pop(p)
                    for sub in range(2):
                        h = 2 * p + sub
                        oT = p3oT.tile([65, 512], F32, tag="oT", name="oT")
                        # PSUM accumulation is per-element has_written; the
                        # first call spans the whole bank so one start/stop
                        # pair suffices for the wide streaming calls.
                        for s in range(NSEC):
                            for cj in range(QT):
                                n = (4 - cj) * 128
                                off = s * SECW + CUM[cj]
                                nc.tensor.matmul(
                                    oT[:, cj * 128:512],
                                    vS[s * 4 + cj][:, h, :],
                                    sexp[:, sub, off:off + n],
                                    start=(s == 0 and cj == 0),
                                    stop=(s == NSEC - 1 and cj == QT - 1))
                        # softmax normalize + drain
                        sums = p3sm.tile([1, TQ], F32, tag="sums", name="sums")
                        nc.vector.tensor_copy(sums[:], oT[64:65, :])
                        nc.vector.reciprocal(sums[:], sums[:])
                        rb = p3sm.tile([128, TQ], F32, tag="rb", name="rb")
                        nc.gpsimd.partition_broadcast(rb[:], sums[:])
                        rbh = rb[sub * 64:sub * 64 + 64, :]
                        dst = oTn[p][sub * 64:sub * 64 + 64, :, :] \
                            .rearrange("p a b -> p (a b)")
                        nc.vector.tensor_copy(dst, oT[0:64, :])
                        nc.vector.tensor_tensor(dst, dst, rbh, ALU.mult)

                for p in range(NP):
                    emit_kproj(p)
                    emit_scores(p)
                    if debug_taps and p == 0:
                        nc.sync.dma_start(taps["sexp0"][:], sexp_of[0][:])
                    if p >= 1:
                        emit_av(p - 1)
                emit_av(NP - 1)
                if debug_taps:
                    nc.sync.dma_start(taps["kT0"][:], kT[0][:])
                    nc.sync.dma_start(taps["qT0"][:], qT[0][:])
                    nc.sync.dma_start(taps["vS0"][:], vS[0][:])
                    nc.sync.dma_start(taps["oTn0"][:], oTn[0][:])

        # ---------- phases 4-5 ----------
        with tc.tile_pool(name="x2p", bufs=1) as x2p, \
             tc.tile_pool(name="p5w", bufs=24) as p5w:
            x2 = [x2p.tile([128, E], F32, tag=f"x2{tq}", name=f"x2{tq}")
                  for tq in range(QT)]
            w1_tiles = {}
            FBLK = 4  # f-tiles per W1 block

            def load_w1_block(fb):
                for ec in range(EC):
                    t = p5w.tile([128, FBLK * 128], BF16, tag="w1c", name="w1c")
                    nc.sync.dma_start(
                        t[:], w1[ec * 128:(ec + 1) * 128,
                                 fb * FBLK * 128:(fb + 1) * FBLK * 128])
                    w1_tiles[(fb, ec)] = t

            load_w1_block(0)
            load_w1_block(1)

            # ---------- phase 4: proj + residual + LN2 + transpose ----------
            with tc.tile_pool(name="p4xr", bufs=2) as p4xr, \
                 tc.tile_pool(name="p4ps", bufs=3, space="PSUM") as p4ps, \
                 tc.tile_pool(name="pT4", bufs=2, space="PSUM") as pT4, \
                 tc.tile_pool(name="p4s", bufs=4) as p4s, \
                 tc.tile_pool(name="p4h", bufs=3) as p4h:
                for tq in range(QT):
                    xr = p4xr.tile([128, E], F32, tag="xr", name="xr")
                    nc.sync.dma_start(xr[:], x_res[tq * 128:(tq + 1) * 128, :])
                    for half in range(2):
                        ps = p4ps.tile([128, 512], F32, tag="pproj", name="pproj")
                        for p in range(NP):
                            nc.tensor.matmul(ps[:], oTn[p][:, tq, :],
                                             wp_sb[p][:, half * 512:(half + 1) * 512],
                                             start=(p == 0), stop=(p == NP - 1))
                        nc.vector.tensor_add(x2[tq][:, half * 512:(half + 1) * 512],
                                             ps[:], xr[:, half * 512:(half + 1) * 512])
                    hnat = p4h.tile([128, E], BF16, tag="hnat", name="hnat")
                    _layernorm(nc, p4s, x2[tq][:], eps_sb, hnat[:], E)
                    pt = pT4.tile([128, EC, 128], BF16, tag="pt", name="pt")
                    for ec in range(EC):
                        nc.tensor.transpose(pt[:, ec, :],
                                            hnat[:, ec * 128:(ec + 1) * 128],
                                            ident_sb[:])
                    nc.scalar.copy(h2A[:, :, tq * 128:(tq + 1) * 128], pt[:])
                if apply_beta2:
                    for ec in range(EC):
                        nc.vector.tensor_scalar_add(h2A[:, ec, :], h2A[:, ec, :],
                                                    be2_sb[:, ec:ec + 1])

            # ---------- phase 5: FFN ----------
            with tc.tile_pool(name="p5g", bufs=1) as p5g, \
                 tc.tile_pool(name="p5w2", bufs=16) as p5w2, \
                 tc.tile_pool(name="p5pg", bufs=3, space="PSUM") as p5pg, \
                 tc.tile_pool(name="p5py", bufs=4, space="PSUM") as p5py, \
                 tc.tile_pool(name="p5o", bufs=3) as p5o:
                gT = [p5g.tile([128, TQ], BF16, tag=f"gT{f}", name=f"gT{f}")
                      for f in range(FT)]
                for fb in range(FT // FBLK):
                    for fi in range(FBLK):
                        f = fb * FBLK + fi
                        ps = p5pg.tile([128, TQ], F32, tag="pg", name="pg")
                        for ec in range(EC):
                            nc.tensor.matmul(
                                ps[:], w1_tiles[(fb, ec)][:, fi * 128:(fi + 1) * 128],
                                h2A[:, ec, :], start=(ec == 0), stop=(ec == EC - 1))
                        nc.scalar.activation(gT[f][:], ps[:], AF.Relu,
                                             bias=b1_sb[:, f:f + 1])
                    if fb + 2 < FT // FBLK and fb + 2 >= 2:
                        load_w1_block(fb + 2)
                for half in range(2):
                    psy = [p5py.tile([128, 512], F32, tag="py", name="py")
                           for _ in range(QT)]
                    for f in range(FT):
                        w2c = p5w2.tile([128, 512], BF16, tag="w2c", name="w2c")
                        nc.sync.dma_start(
                            w2c[:], w2[f * 128:(f + 1) * 128,
                                       half * 512:(half + 1) * 512])
                        for tq in range(QT):
                            nc.tensor.matmul(psy[tq][:],
                                             gT[f][:, tq * 128:(tq + 1) * 128],
                                             w2c[:], start=(f == 0),
                                             stop=(f == FT - 1))
                    for tq in range(QT):
                        outsb = p5o.tile([128, 512], F32, tag="outsb", name="outsb")
                        nc.vector.tensor_add(outsb[:], psy[tq][:],
                                             x2[tq][:, half * 512:(half + 1) * 512])
                        nc.sync.dma_start(
                            y[tq * 128:(tq + 1) * 128, half * 512:(half + 1) * 512],
                            outsb[:])

    nc.compile()
    return nc


_MODULE_CACHE = {}


def _get_module(key=(False, False)):
    if key not in _MODULE_CACHE:
        _MODULE_CACHE[key] = build_module(apply_beta1=key[0], apply_beta2=key[1])
    return _MODULE_CACHE[key]


def make_core_inputs(x, Wq, Wk, Wv, Wproj, bproj, W1, b1, W2, b2, g1, be1, g2, be2):
    """Host-side sharding/folding. Returns (in_maps, meta)."""
    bf = ml_dtypes.bfloat16
    x = np.asarray(x, np.float32)
    g1 = np.asarray(g1, np.float32)
    g2 = np.asarray(g2, np.float32)
    be1 = np.asarray(be1, np.float32)
    be2 = np.asarray(be2, np.float32)
    assert np.all(g1 != 0) and np.all(g2 != 0), "zero LN gamma unsupported"
    apply_beta1 = bool(np.any(be1 != 0))
    apply_beta2 = bool(np.any(be2 != 0))
    be1_eff = (be1 / g1).reshape(EC, 128).T.copy()
    be2_eff = (be2 / g2).reshape(EC, 128).T.copy()

    # lhsT layouts [E, (h, d)] with g folded into rows
    wq_l = (g1[:, None] * np.transpose(Wq, (1, 0, 2)).reshape(E, E)).astype(bf)
    wk_l = (g1[:, None] * np.transpose(Wk, (1, 0, 2)).reshape(E, E)).astype(bf)
    wv_l = (g1[:, None] * np.transpose(Wv, (1, 0, 2)).reshape(E, E)).astype(bf)
    wp_r = np.asarray(Wproj, np.float32).astype(bf)
    w1_l = (g2[:, None] * np.asarray(W1, np.float32)).astype(bf)
    w2_r = np.asarray(W2, np.float32).astype(bf)
    b1c = np.asarray(b1, np.float32).reshape(FT, 128).T.copy()

    in_maps = []
    for c in range(NCORES):
        b, own = c // 4, c % 4
        secs = [(own + s) % 4 for s in range(4)]
        x_allv = np.concatenate([x[b, sig::4, :] for sig in secs], axis=0)
        x_allv = np.ascontiguousarray(x_allv, np.float32)
        x_ownv = x[b, own::4, :]
        x_resid = x_ownv + np.asarray(bproj, np.float32)[None, :]
        # mask[r, s, q] = 1 if q >= r + (sigma(s) > own)  (rotated slots)
        r = np.arange(128)[:, None, None]
        sm = np.array(secs)[None, :, None]
        q = np.arange(128)[None, None, :]
        mask = (q >= r + (sm > own)).astype(bf)
        in_maps.append({
            "x_all": x_allv, "x_res": x_resid.astype(np.float32),
            "wq": wq_l, "wk": wk_l, "wv": wv_l, "wp": wp_r,
            "w1": w1_l, "w2": w2_r, "b1c": b1c,
            "be1c": be1_eff.astype(np.float32),
            "be2c": be2_eff.astype(np.float32),
            "maskd": np.ascontiguousarray(mask),
            "identd": np.eye(128, dtype=bf),
        })
    return in_maps, (apply_beta1, apply_beta2)


def assemble_output(results, b2):
    out = np.empty((B, T, E), np.float32)
    b2 = np.asarray(b2, np.float32)
    for c in range(NCORES):
        b, own = c // 4, c % 4
        out[b, own::4, :] = results[c]["y"] + b2[None, :]
    return out


def kernel(**inputs) -> np.ndarray:
    in_maps, beta_key = make_core_inputs(**inputs)
    nc = _get_module(beta_key)
    res = run_bass_kernel_spmd(nc, in_maps, core_ids=list(range(NCORES)))
    return assemble_output(res.results, inputs["b2"])
